# revision 27
# baseline (speedup 1.0000x reference)
"""CrossAttentionOutLayer Trainium2 kernel.

Math: reference computes, per batch b:
    q = rna @ Wq.T + bq                [n, h*dk]
    k = prot @ Wk.T + bk               [m, h*dk]
    logits[h] = (q_h*scale + rel_h) @ k_h.T
    out = mean_h logits                [n, m]

The head-mean of per-head inner products collapses into one flat inner
product over the h*dk=512 axis:
    out[i,j] = (scale/H * q[i,:] + rel_flat/H) . k[j,:]
so with Wq2 = (scale/H)*Wq, bq2 = (scale/H)*bq + rel_flat/H:
    out = (rna @ Wq2.T + bq2) @ (prot @ Wk.T + bk).T
Three GEMMs per batch. Data-parallel: batch b -> core b (8 cores).

Schedule notes (tuned against the cost-model timeline, verified on HW):
- activations ship host-transposed (feature-major) in bf16; all GEMMs
  bf16 with fp32 PSUM accumulation.
- the PE clock p-state ramp is burned by ~free 1-column warmup matmuls
  so every real matmul runs at full clock.
- input DMAs are ordered by PE need; the first DMA is a packed
  [wk tile0 | first half of prot tile0] so the first matmul gates on a
  single 256KB transfer. The HWDGE descriptor-issue engine is shared
  across queues (~630ns per DMA), so activation tiles later in the
  consumption order ride grouped DMAs.
- GEMM2 is contraction-outer (gates tile-by-tile on DMA arrivals);
  GEMM1 runs banks 0-1 contraction-outer first (tolerates the last
  Q-side DMAs still in flight), then banks 2-7 contraction-inner so
  their stops stagger ahead of GEMM3.
- psum drains (bias adds / output copies) split across DVE and ACT;
  output stores ride the otherwise-idle SP HWDGE queue (Pool/SWDGE
  issue costs ~1us per store; ACT's sequencer is kept off the store
  path so its copies stay prompt).
"""

import os
from contextlib import ExitStack

import numpy as np
import ml_dtypes

# 1 = (timing probe) repeat-loop wraps only the matmul/bias phases; DMAs
# and output copies/stores run once outside the loop
_LOOPMM = os.environ.get("KERNEL_LOOPMM", "0") == "1"
# number of 1-column warmup matmuls issued before the real work
_WARMUP = int(os.environ.get("KERNEL_WARMUP", "32"))
# number of 128-column warmup matmuls after the 1-column ones. Measured
# on HW: these DELAY the real work (~240ns each on the critical path) —
# the hardware head is shorter than the cost model's; keep at 0.
_WARMUP_BIG = int(os.environ.get("KERNEL_WARMUP_BIG", "0"))

import concourse.bass as bass
import concourse.bacc as bacc
import concourse.tile as tile
import concourse.mybir as mybir
from concourse import bass_utils
from concourse.bass import ts

B, N, M = 8, 1024, 1024
DIM2 = 1280            # rna in-features  = 10*128
KIN = 1344             # protein in-features
KINP = 1408            # padded to 11*128
F = 512                # h*dk flat feature dim = 4*128
H, DK = 8, 64
SCALE = DK ** -0.5
NCORES = 8

NQ = DIM2 // 128       # 10 contraction tiles for Q gemm
NK = KINP // 128       # 11 contraction tiles for K gemm
NF = F // 128          # 4 feature tiles
NB = N // 128          # 8 row blocks of output
NMC = M // 512         # 2 column chunks of output

WK_CHUNKS = [1, 1, 2, 4, 3]  # contraction tiles per wk-load DMA (chunk0 = w0)
WQ_CHUNKS = [1, 3, 4, 2]     # contraction tiles per wq-load DMA
XQ_GROUPS = [3, 2, 2, 2, 1]  # rna tiles per act-load DMA

BF16 = mybir.dt.bfloat16
F32 = mybir.dt.float32

_CACHE = {}


def _build_program(reps=1):
    nc = bacc.Bacc(
        "TRN2", target_bir_lowering=False, debug=False, num_devices=NCORES
    )

    # w0 packs [wk tile0 (128x512) | prot tile0 cols 0-512] so the first
    # matmul is gated by one 256KB transfer in the first HWDGE slot.
    w0_d = nc.dram_tensor("w0", [128, 1024], BF16, kind="ExternalInput").ap()
    rna_d = nc.dram_tensor("rna", [DIM2, N], BF16, kind="ExternalInput").ap()
    prot_d = nc.dram_tensor("prot", [KINP, M], BF16, kind="ExternalInput").ap()
    wq_d = nc.dram_tensor("wqt", [DIM2, F], BF16, kind="ExternalInput").ap()
    wk_d = nc.dram_tensor("wkt", [KINP, F], BF16, kind="ExternalInput").ap()
    b2_d = nc.dram_tensor("b2", [128, 2 * NF], F32, kind="ExternalInput").ap()
    out_d = nc.dram_tensor("out", [N, M], BF16, kind="ExternalOutput").ap()

    with tile.TileContext(nc) as tc:
        with (
            tc.tile_pool(name="weights", bufs=1) as wpool,
            tc.tile_pool(name="acts", bufs=1) as apool,
            tc.tile_pool(name="qk", bufs=1) as qkpool,
            tc.tile_pool(name="bias", bufs=1) as bpool,
            tc.tile_pool(name="outs", bufs=4) as opool,
            tc.tile_pool(name="psum", bufs=1, space="PSUM") as pspool,
            ExitStack() as loop_ctx,
        ):
            if reps > 1 and not _LOOPMM:
                loop_ctx.enter_context(
                    tc.For_i(0, reps, 1, hint_engines=(mybir.EngineType.PE,))
                )
            # ---- persistent SBUF tensors ----
            w0_t = wpool.tile([128, 2, 512], BF16, tag="w0", name="w0sb")
            wk_c = [w0_t] + [
                wpool.tile([128, sz, F], BF16, tag=f"wkc{j}", name=f"wkc{j}")
                for j, sz in enumerate(WK_CHUNKS[1:], start=1)
            ]
            wq_c = [
                wpool.tile([128, sz, F], BF16, tag=f"wqc{j}", name=f"wqc{j}")
                for j, sz in enumerate(WQ_CHUNKS)
            ]

            def chunk_slice(chunks, tiles, i):
                """[128, F] weight view for global contraction tile i."""
                j = 0
                while i >= chunks[j]:
                    i -= chunks[j]
                    j += 1
                return tiles[j][:, i]

            # prot tile0 second half; prot tiles 1-10 individually
            xk0b = apool.tile([128, 512], BF16, tag="xk0b", name="xk0b")
            xk_r = [
                apool.tile([128, M], BF16, tag=f"xk{i}", name=f"xk{i}")
                for i in range(1, NK)
            ]
            xq_g = [
                apool.tile([128, g, N], BF16, tag=f"xqg{j}", name=f"xqg{j}")
                for j, g in enumerate(XQ_GROUPS)
            ]
            xq_t = [xq_g[j][:, l] for j, g in enumerate(XQ_GROUPS)
                    for l in range(g)]
            kt_t = [
                qkpool.tile([128, M], BF16, tag=f"kt{f}", name=f"kt{f}")
                for f in range(NF)
            ]
            q2_t = [
                qkpool.tile([128, N], BF16, tag=f"q2{f}", name=f"q2{f}")
                for f in range(NF)
            ]
            b2_t = bpool.tile([128, 2 * NF], F32, tag="b2", name="b2sb")

            # ---- PE warmup: 1-col matmuls on a memset tile ----
            # These retire in a few ns each and burn the tensor engine's
            # slow p-state window before the first real matmul.
            if _WARMUP or _WARMUP_BIG:
                wu = bpool.tile([128, 128], BF16, tag="wu", name="wu")
                nc.vector.memset(wu, 0.0)
                ps_wu = pspool.tile([128, 512], F32, tag="ps7", name="ps_wu")
                for _ in range(_WARMUP):
                    nc.tensor.matmul(
                        ps_wu[:, 0:1], wu, wu[:, 0:1], start=True, stop=True
                    )
                for _ in range(_WARMUP_BIG):
                    nc.tensor.matmul(
                        ps_wu[:, 0:128], wu, wu, start=True, stop=True
                    )

            # ---- DMA issue plan (ordered by PE need) ----
            def load_wchunk(eng, w_c, w_d, chunks, j):
                off = sum(chunks[:j])
                src = w_d[off * 128 : (off + chunks[j]) * 128, :]
                eng.dma_start(w_c[j], src.rearrange("(t p) f -> p t f", p=128))

            def load_xk(i):
                nc.sync.dma_start(xk_r[i - 1], prot_d[ts(i, 128), :])

            def load_xqg(j):
                off = sum(XQ_GROUPS[:j])
                cnt = XQ_GROUPS[j]
                src = rna_d[off * 128 : (off + cnt) * 128, :]
                nc.sync.dma_start(
                    xq_g[j], src.rearrange("(t p) m -> p t m", p=128)
                )

            # The DMA engine round-robins between the SP and ACT queues, so
            # the global transfer order ~= this interleaving: all prot (xk)
            # tiles land before the rna (xq) groups, weights slot between.
            nc.sync.dma_start(
                w0_t, w0_d.rearrange("p (t f) -> p t f", t=2)
            )
            nc.scalar.dma_start(xk0b, prot_d[0:128, 512:1024])
            load_xk(1)
            load_wchunk(nc.scalar, wk_c, wk_d, WK_CHUNKS, 1)
            load_xk(2)
            load_wchunk(nc.scalar, wk_c, wk_d, WK_CHUNKS, 2)
            load_xk(3)
            load_wchunk(nc.scalar, wk_c, wk_d, WK_CHUNKS, 3)
            load_xk(4)
            load_wchunk(nc.scalar, wk_c, wk_d, WK_CHUNKS, 4)
            load_xk(5)
            load_wchunk(nc.scalar, wq_c, wq_d, WQ_CHUNKS, 0)
            load_xk(6)
            nc.scalar.dma_start(b2_t, b2_d)
            load_xk(7)
            load_wchunk(nc.scalar, wq_c, wq_d, WQ_CHUNKS, 1)
            load_xk(8)
            load_wchunk(nc.scalar, wq_c, wq_d, WQ_CHUNKS, 2)
            load_xk(9)
            load_wchunk(nc.scalar, wq_c, wq_d, WQ_CHUNKS, 3)
            load_xk(10)
            load_xqg(0)
            load_xqg(1)
            load_xqg(2)
            load_xqg(3)
            load_xqg(4)

            if reps > 1 and _LOOPMM:
                loop_ctx.enter_context(
                    tc.For_i(0, reps, 1, hint_engines=(mybir.EngineType.PE,))
                )

            def xk_half(i, mc):
                if i == 0:
                    return w0_t[:, 1] if mc == 0 else xk0b
                return xk_r[i - 1][:, ts(mc, 512)]

            # ---- GEMM2: kT[f,m] = sum_i WkT[i,f].T @ protT[i,m]  (+bk) ----
            # contraction-outer: each i-group is gated on one DMA arrival
            ps_k = [
                pspool.tile([128, 512], F32, tag=f"ps{j}", name=f"psk{j}")
                for j in range(8)
            ]
            for i in range(NK):
                wki = chunk_slice(WK_CHUNKS, wk_c, i)
                if i == 0:
                    # mc-outer: first 4 matmuls need only w0 (prot half 0)
                    order = [(f, mc) for mc in range(NMC) for f in range(NF)]
                else:
                    order = [(f, mc) for f in range(NF) for mc in range(NMC)]
                for f, mc in order:
                    nc.tensor.matmul(
                        ps_k[f * NMC + mc],
                        wki[:, ts(f, 128)],
                        xk_half(i, mc),
                        start=(i == 0),
                        stop=(i == NK - 1),
                    )
            # drain psum on both DVE and ACT so the banks free ~2x faster
            for f in range(NF):
                for mc in range(NMC):
                    j = f * NMC + mc
                    dst = kt_t[f][:, ts(mc, 512)]
                    if j % 2 == 0:
                        nc.vector.tensor_scalar_add(
                            dst, ps_k[j], b2_t[:, f : f + 1]
                        )
                    else:
                        nc.scalar.add(dst, ps_k[j], b2_t[:, f : f + 1])

            # ---- GEMM1: q2T[f,n] = sum_i WqT[i,f].T @ rnaT[i,n]  (+bq2) ----
            ps_q = [
                pspool.tile([128, 512], F32, tag=f"ps{j}", name=f"psq{j}")
                for j in range(8)
            ]
            # f-outer / contraction-mid / nc-inner: each weight tile slice
            # feeds 2 consecutive matmuls (amortizes the PE weight load);
            # f=0's i-loop still gates tile-by-tile on the late Q-side DMAs,
            # and each f's two banks stop staggered ahead of GEMM3.
            for f in range(NF):
                for i in range(NQ):
                    wqif = chunk_slice(WQ_CHUNKS, wq_c, i)[:, ts(f, 128)]
                    for nc_ in range(NMC):
                        nc.tensor.matmul(
                            ps_q[f * NMC + nc_],
                            wqif,
                            xq_t[i][:, ts(nc_, 512)],
                            start=(i == 0),
                            stop=(i == NQ - 1),
                        )
            for f in range(NF):
                for nc_ in range(NMC):
                    j = f * NMC + nc_
                    dst = q2_t[f][:, ts(nc_, 512)]
                    if j % 2 == 0:
                        nc.vector.tensor_scalar_add(
                            dst, ps_q[j], b2_t[:, NF + f : NF + f + 1]
                        )
                    else:
                        nc.scalar.add(
                            dst, ps_q[j], b2_t[:, NF + f : NF + f + 1]
                        )

            # ---- GEMM3: out[n,m] = sum_f q2T[f,n].T @ kT[f,m] ----
            # f-mid / mc-inner: each q2 weight slice feeds 2 consecutive
            # matmuls; both mc-banks of a row block finish together, so
            # their copies pair up across ACT+DVE.
            for nb in range(NB):
                ps_row = [
                    pspool.tile(
                        [128, 512],
                        F32,
                        tag=f"ps{(nb * NMC + mc) % 8}",
                        name=f"ps3_{nb}_{mc}",
                    )
                    for mc in range(NMC)
                ]
                for f in range(NF):
                    q2f = q2_t[f][:, ts(nb, 128)]
                    for mc in range(NMC):
                        nc.tensor.matmul(
                            ps_row[mc],
                            q2f,
                            kt_t[f][:, ts(mc, 512)],
                            start=(f == 0),
                            stop=(f == NF - 1),
                        )
                if _LOOPMM and reps > 1:
                    continue  # timing probe: skip drain of GEMM3 psums
                for mc in range(NMC):
                    ot = opool.tile(
                        [128, 512], BF16, tag="ot", name=f"ot{nb}_{mc}"
                    )
                    # the two banks of a row finish together: pair the
                    # copies across ACT (mc=0) and DVE (mc=1)
                    if mc == 0:
                        nc.scalar.activation(
                            ot, ps_row[mc], mybir.ActivationFunctionType.Copy
                        )
                    else:
                        nc.vector.tensor_copy(ot, ps_row[mc])
                    # all stores on the SP HWDGE queue: SP is idle during
                    # GEMM3 and its DGE-config time (565ns) fits under the
                    # 852ns/tile production rate; keeping stores off ACT's
                    # sequencer keeps the ACT psum-copies prompt.
                    nc.sync.dma_start(out_d[ts(nb, 128), ts(mc, 512)], ot)

    nc.compile()
    return nc


def _get_program(reps=1):
    key = ("nc", reps)
    if key not in _CACHE:
        _CACHE[key] = _build_program(reps)
    return _CACHE[key]


def _prep_inputs(rna_reps, protein_reps, Wq, bq, Wk, bk, rel_bias):
    bf16 = ml_dtypes.bfloat16
    # fold scale/H into Wq; fold rel_bias head-mean into the q bias
    rel_flat = np.asarray(rel_bias, np.float32).reshape(H * DK)
    wq2t = (np.asarray(Wq, np.float32).T * (SCALE / H)).astype(bf16)  # [DIM2,F]
    bq2 = (SCALE / H) * np.asarray(bq, np.float32) + rel_flat / H
    wkt = np.zeros((KINP, F), dtype=bf16)
    wkt[:KIN] = np.asarray(Wk, np.float32).T.astype(bf16)
    bk2 = np.asarray(bk, np.float32)

    # packed biases: col f -> bk chunk f, col NF+f -> bq chunk f
    b2 = np.empty((128, 2 * NF), np.float32)
    for f in range(NF):
        b2[:, f] = bk2[f * 128 : (f + 1) * 128]
        b2[:, NF + f] = bq2[f * 128 : (f + 1) * 128]

    # feature-major layout: [B, D, tokens]
    rna_bf = (
        np.asarray(rna_reps, np.float32).transpose(0, 2, 1).astype(bf16)
    )
    prot_bf = np.zeros((B, KINP, M), dtype=bf16)
    prot_bf[:, :KIN] = (
        np.asarray(protein_reps, np.float32).transpose(0, 2, 1).astype(bf16)
    )

    in_maps = []
    for b in range(B):
        w0 = np.concatenate(
            [np.asarray(wkt[0:128, :]), prot_bf[b][0:128, 0:512]], axis=1
        )
        in_maps.append(
            {
                "w0": np.ascontiguousarray(w0),
                "rna": np.ascontiguousarray(rna_bf[b]),
                "prot": np.ascontiguousarray(prot_bf[b]),
                "wqt": wq2t,
                "wkt": wkt,
                "b2": b2,
            }
        )
    return in_maps


def kernel(rna_reps, protein_reps, Wq, bq, Wk, bk, rel_bias, **_ignored):
    in_maps = _prep_inputs(rna_reps, protein_reps, Wq, bq, Wk, bk, rel_bias)
    nc = _get_program()
    res = bass_utils.run_bass_kernel_spmd(
        nc, in_maps, core_ids=list(range(NCORES))
    )
    out = np.stack(
        [np.asarray(res.results[b]["out"], np.float32) for b in range(B)], axis=0
    )
    return out


# revision 29
# speedup vs baseline: 1.0525x; 1.0525x over previous
"""CrossAttentionOutLayer Trainium2 kernel.

Math: reference computes, per batch b:
    q = rna @ Wq.T + bq                [n, h*dk]
    k = prot @ Wk.T + bk               [m, h*dk]
    logits[h] = (q_h*scale + rel_h) @ k_h.T
    out = mean_h logits                [n, m]

The head-mean of per-head inner products collapses into one flat inner
product over the h*dk=512 axis:
    out[i,j] = (scale/H * q[i,:] + rel_flat/H) . k[j,:]
so with Wq2 = (scale/H)*Wq, bq2 = (scale/H)*bq + rel_flat/H:
    out = (rna @ Wq2.T + bq2) @ (prot @ Wk.T + bk).T
Three GEMMs per batch. Data-parallel: batch b -> core b (8 cores).

Schedule notes (tuned against the cost-model timeline, verified on HW):
- activations ship host-transposed (feature-major) in bf16; all GEMMs
  bf16 with fp32 PSUM accumulation.
- the PE clock p-state ramp is burned by ~free 1-column warmup matmuls
  so every real matmul runs at full clock.
- input DMAs are ordered by PE need; the first DMA is a packed
  [wk tile0 | first half of prot tile0] so the first matmul gates on a
  single 256KB transfer. The HWDGE descriptor-issue engine is shared
  across queues (~630ns per DMA), so activation tiles later in the
  consumption order ride grouped DMAs.
- GEMM2 is contraction-outer (gates tile-by-tile on DMA arrivals);
  GEMM1 runs banks 0-1 contraction-outer first (tolerates the last
  Q-side DMAs still in flight), then banks 2-7 contraction-inner so
  their stops stagger ahead of GEMM3.
- psum drains (bias adds / output copies) split across DVE and ACT;
  output stores ride the otherwise-idle SP HWDGE queue (Pool/SWDGE
  issue costs ~1us per store; ACT's sequencer is kept off the store
  path so its copies stay prompt).
"""

import os
from contextlib import ExitStack

import numpy as np
import ml_dtypes

# 1 = (timing probe) repeat-loop wraps only the matmul/bias phases; DMAs
# and output copies/stores run once outside the loop
_LOOPMM = os.environ.get("KERNEL_LOOPMM", "0") == "1"
# number of 1-column warmup matmuls issued before the real work
_WARMUP = int(os.environ.get("KERNEL_WARMUP", "32"))
# number of 128-column warmup matmuls after the 1-column ones. Measured
# on HW: these DELAY the real work (~240ns each on the critical path) —
# the hardware head is shorter than the cost model's; keep at 0.
_WARMUP_BIG = int(os.environ.get("KERNEL_WARMUP_BIG", "0"))
# 1 = load w0 via Pool/SWDGE instead of SP HWDGE (loop-prefetch experiment)
_W0POOL = os.environ.get("KERNEL_W0POOL", "0") == "1"

import concourse.bass as bass
import concourse.bacc as bacc
import concourse.tile as tile
import concourse.mybir as mybir
from concourse import bass_utils
from concourse.bass import ts

B, N, M = 8, 1024, 1024
DIM2 = 1280            # rna in-features  = 10*128
KIN = 1344             # protein in-features
KINP = 1408            # padded to 11*128
F = 512                # h*dk flat feature dim = 4*128
H, DK = 8, 64
SCALE = DK ** -0.5
NCORES = 8

NQ = DIM2 // 128       # 10 contraction tiles for Q gemm
NK = KINP // 128       # 11 contraction tiles for K gemm
NF = F // 128          # 4 feature tiles
NB = N // 128          # 8 row blocks of output
NMC = M // 512         # 2 column chunks of output

WK_CHUNKS = [1, 1, 2, 4, 3]  # contraction tiles per wk-load DMA (chunk0 = w0)
WQ_CHUNKS = [1, 3, 4, 2]     # contraction tiles per wq-load DMA
XQ_GROUPS = [3, 2, 2, 2, 1]  # rna tiles per act-load DMA

BF16 = mybir.dt.bfloat16
F32 = mybir.dt.float32

_CACHE = {}


def _build_program(reps=1):
    nc = bacc.Bacc(
        "TRN2", target_bir_lowering=False, debug=False, num_devices=NCORES
    )

    # w0 packs [wk tile0 (128x512) | prot tile0 cols 0-512] so the first
    # matmul is gated by one 256KB transfer in the first HWDGE slot.
    w0_d = nc.dram_tensor("w0", [128, 1024], BF16, kind="ExternalInput").ap()
    rna_d = nc.dram_tensor("rna", [DIM2, N], BF16, kind="ExternalInput").ap()
    prot_d = nc.dram_tensor("prot", [KINP, M], BF16, kind="ExternalInput").ap()
    wq_d = nc.dram_tensor("wqt", [DIM2, F], BF16, kind="ExternalInput").ap()
    wk_d = nc.dram_tensor("wkt", [KINP, F], BF16, kind="ExternalInput").ap()
    b2_d = nc.dram_tensor("b2", [128, 2 * NF], F32, kind="ExternalInput").ap()
    out_d = nc.dram_tensor("out", [N, M], BF16, kind="ExternalOutput").ap()

    with tile.TileContext(nc) as tc:
        with (
            tc.tile_pool(name="weights", bufs=1) as wpool,
            tc.tile_pool(name="acts", bufs=1) as apool,
            tc.tile_pool(name="qk", bufs=1) as qkpool,
            tc.tile_pool(name="bias", bufs=1) as bpool,
            tc.tile_pool(name="outs", bufs=4) as opool,
            tc.tile_pool(name="psum", bufs=1, space="PSUM") as pspool,
            ExitStack() as loop_ctx,
        ):
            if reps > 1 and not _LOOPMM:
                loop_ctx.enter_context(
                    tc.For_i(0, reps, 1, hint_engines=(mybir.EngineType.PE,))
                )
            # ---- persistent SBUF tensors ----
            w0_t = wpool.tile([128, 2, 512], BF16, tag="w0", name="w0sb")
            wk_c = [w0_t] + [
                wpool.tile([128, sz, F], BF16, tag=f"wkc{j}", name=f"wkc{j}")
                for j, sz in enumerate(WK_CHUNKS[1:], start=1)
            ]
            wq_c = [
                wpool.tile([128, sz, F], BF16, tag=f"wqc{j}", name=f"wqc{j}")
                for j, sz in enumerate(WQ_CHUNKS)
            ]

            def chunk_slice(chunks, tiles, i):
                """[128, F] weight view for global contraction tile i."""
                j = 0
                while i >= chunks[j]:
                    i -= chunks[j]
                    j += 1
                return tiles[j][:, i]

            # prot tile0 second half; prot tiles 1-10 individually
            xk0b = apool.tile([128, 512], BF16, tag="xk0b", name="xk0b")
            xk_r = [
                apool.tile([128, M], BF16, tag=f"xk{i}", name=f"xk{i}")
                for i in range(1, NK)
            ]
            xq_g = [
                apool.tile([128, g, N], BF16, tag=f"xqg{j}", name=f"xqg{j}")
                for j, g in enumerate(XQ_GROUPS)
            ]
            xq_t = [xq_g[j][:, l] for j, g in enumerate(XQ_GROUPS)
                    for l in range(g)]
            kt_t = [
                qkpool.tile([128, M], BF16, tag=f"kt{f}", name=f"kt{f}")
                for f in range(NF)
            ]
            q2_t = [
                qkpool.tile([128, N], BF16, tag=f"q2{f}", name=f"q2{f}")
                for f in range(NF)
            ]
            b2_t = bpool.tile([128, 2 * NF], F32, tag="b2", name="b2sb")

            # ---- PE warmup: 1-col matmuls on a memset tile ----
            # These retire in a few ns each and burn the tensor engine's
            # slow p-state window before the first real matmul.
            if _WARMUP or _WARMUP_BIG:
                wu = bpool.tile([128, 128], BF16, tag="wu", name="wu")
                nc.vector.memset(wu, 0.0)
                ps_wu = pspool.tile([128, 512], F32, tag="ps7", name="ps_wu")
                for _ in range(_WARMUP):
                    nc.tensor.matmul(
                        ps_wu[:, 0:1], wu, wu[:, 0:1], start=True, stop=True
                    )
                for _ in range(_WARMUP_BIG):
                    nc.tensor.matmul(
                        ps_wu[:, 0:128], wu, wu, start=True, stop=True
                    )

            # ---- DMA issue plan (ordered by PE need) ----
            def load_wchunk(eng, w_c, w_d, chunks, j):
                off = sum(chunks[:j])
                src = w_d[off * 128 : (off + chunks[j]) * 128, :]
                eng.dma_start(w_c[j], src.rearrange("(t p) f -> p t f", p=128))

            def load_xk(i):
                nc.sync.dma_start(xk_r[i - 1], prot_d[ts(i, 128), :])

            def load_xqg(j):
                off = sum(XQ_GROUPS[:j])
                cnt = XQ_GROUPS[j]
                src = rna_d[off * 128 : (off + cnt) * 128, :]
                nc.sync.dma_start(
                    xq_g[j], src.rearrange("(t p) m -> p t m", p=128)
                )

            # The DMA engine round-robins between the SP and ACT queues, so
            # the global transfer order ~= this interleaving: all prot (xk)
            # tiles land before the rna (xq) groups, weights slot between.
            (nc.gpsimd if _W0POOL else nc.sync).dma_start(
                w0_t, w0_d.rearrange("p (t f) -> p t f", t=2)
            )
            nc.scalar.dma_start(xk0b, prot_d[0:128, 512:1024])
            load_xk(1)
            load_wchunk(nc.scalar, wk_c, wk_d, WK_CHUNKS, 1)
            load_xk(2)
            load_wchunk(nc.scalar, wk_c, wk_d, WK_CHUNKS, 2)
            load_xk(3)
            load_wchunk(nc.scalar, wk_c, wk_d, WK_CHUNKS, 3)
            load_xk(4)
            load_wchunk(nc.scalar, wk_c, wk_d, WK_CHUNKS, 4)
            load_xk(5)
            load_wchunk(nc.scalar, wq_c, wq_d, WQ_CHUNKS, 0)
            load_xk(6)
            nc.scalar.dma_start(b2_t, b2_d)
            load_xk(7)
            load_wchunk(nc.scalar, wq_c, wq_d, WQ_CHUNKS, 1)
            load_xk(8)
            load_wchunk(nc.scalar, wq_c, wq_d, WQ_CHUNKS, 2)
            load_xk(9)
            load_wchunk(nc.scalar, wq_c, wq_d, WQ_CHUNKS, 3)
            load_xk(10)
            load_xqg(0)
            load_xqg(1)
            load_xqg(2)
            load_xqg(3)
            load_xqg(4)

            if reps > 1 and _LOOPMM:
                loop_ctx.enter_context(
                    tc.For_i(0, reps, 1, hint_engines=(mybir.EngineType.PE,))
                )

            def xk_half(i, mc):
                if i == 0:
                    return w0_t[:, 1] if mc == 0 else xk0b
                return xk_r[i - 1][:, ts(mc, 512)]

            # ---- GEMM2: kT[f,m] = sum_i WkT[i,f].T @ protT[i,m]  (+bk) ----
            # contraction-outer: each i-group is gated on one DMA arrival
            ps_k = [
                pspool.tile([128, 512], F32, tag=f"ps{j}", name=f"psk{j}")
                for j in range(8)
            ]
            for i in range(NK):
                wki = chunk_slice(WK_CHUNKS, wk_c, i)
                if i == 0:
                    # mc-outer: first 4 matmuls need only w0 (prot half 0)
                    order = [(f, mc) for mc in range(NMC) for f in range(NF)]
                else:
                    order = [(f, mc) for f in range(NF) for mc in range(NMC)]
                for f, mc in order:
                    nc.tensor.matmul(
                        ps_k[f * NMC + mc],
                        wki[:, ts(f, 128)],
                        xk_half(i, mc),
                        start=(i == 0),
                        stop=(i == NK - 1),
                    )
            # drain psum on both DVE and ACT so the banks free ~2x faster
            for f in range(NF):
                for mc in range(NMC):
                    j = f * NMC + mc
                    dst = kt_t[f][:, ts(mc, 512)]
                    if j % 2 == 0:
                        nc.vector.tensor_scalar_add(
                            dst, ps_k[j], b2_t[:, f : f + 1]
                        )
                    else:
                        nc.scalar.add(dst, ps_k[j], b2_t[:, f : f + 1])

            # ---- GEMM1: q2T[f,n] = sum_i WqT[i,f].T @ rnaT[i,n]  (+bq2) ----
            ps_q = [
                pspool.tile([128, 512], F32, tag=f"ps{j}", name=f"psq{j}")
                for j in range(8)
            ]
            # f-outer / contraction-mid / nc-inner: each weight tile slice
            # feeds 2 consecutive matmuls (amortizes the PE weight load);
            # f=0's i-loop still gates tile-by-tile on the late Q-side DMAs,
            # and each f's two banks stop staggered ahead of GEMM3.
            for f in range(NF):
                for i in range(NQ):
                    wqif = chunk_slice(WQ_CHUNKS, wq_c, i)[:, ts(f, 128)]
                    for nc_ in range(NMC):
                        nc.tensor.matmul(
                            ps_q[f * NMC + nc_],
                            wqif,
                            xq_t[i][:, ts(nc_, 512)],
                            start=(i == 0),
                            stop=(i == NQ - 1),
                        )
            for f in range(NF):
                for nc_ in range(NMC):
                    j = f * NMC + nc_
                    dst = q2_t[f][:, ts(nc_, 512)]
                    if j % 2 == 0:
                        nc.vector.tensor_scalar_add(
                            dst, ps_q[j], b2_t[:, NF + f : NF + f + 1]
                        )
                    else:
                        nc.scalar.add(
                            dst, ps_q[j], b2_t[:, NF + f : NF + f + 1]
                        )

            # ---- GEMM3: out[n,m] = sum_f q2T[f,n].T @ kT[f,m] ----
            # f-mid / mc-inner: each q2 weight slice feeds 2 consecutive
            # matmuls; both mc-banks of a row block finish together, so
            # their copies pair up across ACT+DVE.
            for nb in range(NB):
                ps_row = [
                    pspool.tile(
                        [128, 512],
                        F32,
                        tag=f"ps{(nb * NMC + mc) % 8}",
                        name=f"ps3_{nb}_{mc}",
                    )
                    for mc in range(NMC)
                ]
                for f in range(NF):
                    q2f = q2_t[f][:, ts(nb, 128)]
                    for mc in range(NMC):
                        nc.tensor.matmul(
                            ps_row[mc],
                            q2f,
                            kt_t[f][:, ts(mc, 512)],
                            start=(f == 0),
                            stop=(f == NF - 1),
                        )
                if _LOOPMM and reps > 1:
                    continue  # timing probe: skip drain of GEMM3 psums
                for mc in range(NMC):
                    ot = opool.tile(
                        [128, 512], BF16, tag="ot", name=f"ot{nb}_{mc}"
                    )
                    # the two banks of a row finish together: pair the
                    # copies across ACT (mc=0) and DVE (mc=1)
                    if mc == 0:
                        nc.scalar.activation(
                            ot, ps_row[mc], mybir.ActivationFunctionType.Copy
                        )
                    else:
                        nc.vector.tensor_copy(ot, ps_row[mc])
                    # all stores on the SP HWDGE queue: SP is idle during
                    # GEMM3 and its DGE-config time (565ns) fits under the
                    # 852ns/tile production rate; keeping stores off ACT's
                    # sequencer keeps the ACT psum-copies prompt.
                    nc.sync.dma_start(out_d[ts(nb, 128), ts(mc, 512)], ot)

    nc.compile()
    return nc


def _get_program(reps=1):
    key = ("nc", reps)
    if key not in _CACHE:
        _CACHE[key] = _build_program(reps)
    return _CACHE[key]


def _prep_inputs(rna_reps, protein_reps, Wq, bq, Wk, bk, rel_bias):
    bf16 = ml_dtypes.bfloat16
    # fold scale/H into Wq; fold rel_bias head-mean into the q bias
    rel_flat = np.asarray(rel_bias, np.float32).reshape(H * DK)
    wq2t = (np.asarray(Wq, np.float32).T * (SCALE / H)).astype(bf16)  # [DIM2,F]
    bq2 = (SCALE / H) * np.asarray(bq, np.float32) + rel_flat / H
    wkt = np.zeros((KINP, F), dtype=bf16)
    wkt[:KIN] = np.asarray(Wk, np.float32).T.astype(bf16)
    bk2 = np.asarray(bk, np.float32)

    # packed biases: col f -> bk chunk f, col NF+f -> bq chunk f
    b2 = np.empty((128, 2 * NF), np.float32)
    for f in range(NF):
        b2[:, f] = bk2[f * 128 : (f + 1) * 128]
        b2[:, NF + f] = bq2[f * 128 : (f + 1) * 128]

    # feature-major layout: [B, D, tokens]
    rna_bf = (
        np.asarray(rna_reps, np.float32).transpose(0, 2, 1).astype(bf16)
    )
    prot_bf = np.zeros((B, KINP, M), dtype=bf16)
    prot_bf[:, :KIN] = (
        np.asarray(protein_reps, np.float32).transpose(0, 2, 1).astype(bf16)
    )

    in_maps = []
    for b in range(B):
        w0 = np.concatenate(
            [np.asarray(wkt[0:128, :]), prot_bf[b][0:128, 0:512]], axis=1
        )
        in_maps.append(
            {
                "w0": np.ascontiguousarray(w0),
                "rna": np.ascontiguousarray(rna_bf[b]),
                "prot": np.ascontiguousarray(prot_bf[b]),
                "wqt": wq2t,
                "wkt": wkt,
                "b2": b2,
            }
        )
    return in_maps


def kernel(rna_reps, protein_reps, Wq, bq, Wk, bk, rel_bias, **_ignored):
    in_maps = _prep_inputs(rna_reps, protein_reps, Wq, bq, Wk, bk, rel_bias)
    nc = _get_program()
    res = bass_utils.run_bass_kernel_spmd(
        nc, in_maps, core_ids=list(range(NCORES))
    )
    out = np.stack(
        [np.asarray(res.results[b]["out"], np.float32) for b in range(B)], axis=0
    )
    return out
